# revision 10
# baseline (speedup 1.0000x reference)
"""Distributed attention kernel for 8 Trainium2 NeuronCores.

reference:
    query = features_host @ Q          # [4096, 1024]
    key   = features_guests @ K        # [8192, 1024]
    value = features_guests @ V        # [8192, 1024]
    att   = softmax(query @ key.T / 32, axis=1)
    out   = att @ value                # [4096, 1024]

Sharding: host rows (N=4096) split across 8 cores (512 each). Guest rows
(M=8192) split across 8 cores (1024 each) for the key/value projections.
keyT is all-gathered as bf16 in four m-quarter chunks (so the S sweep can
start as soon as the first quarter lands), value in two m-halves; the
attention sweep streams in behind the collective queue
(kq0 kq1 kq2 kq3 v0 v1).

Per-core pipeline (bf16 matmuls, fp32 PSUM accumulation):
  guests/host -> bf16 -> DRAM -> XBAR DMA-transpose -> guestsT / hostT
  keyT halves -> quarter k-AGs; qT; S sweep by m-quarter with the two
  value halves + v-AGs interleaved between quarters; exp on ScalarE
  (scale=1/32) into a persistent bf16 P matrix; rowsum via ones-matmul;
  PV sweep accumulates O[n, 0:1024] across all m in all 8 PSUM banks;
  divide by rowsum (PE-transposed to per-partition) and write out.
"""

import sys

for _p in ("/opt/trn_rl_repo", "/root/.axon_site/_ro/trn_rl_repo"):
    if _p not in sys.path:
        sys.path.insert(0, _p)

import numpy as np

N_HOST = 4096
N_GUEST = 8192
DIM = 1024
N_CORES = 8
N_SH = N_HOST // N_CORES      # 512 host rows per core
M_SH = N_GUEST // N_CORES     # 1024 guest rows per core
P = 128

_CACHE = {}


def _build():
    import concourse.bass as bass  # noqa: F401
    import concourse.mybir as mybir
    import concourse.tile as tile
    from concourse import bacc
    from concourse.masks import make_identity

    f32 = mybir.dt.float32
    bf16 = mybir.dt.bfloat16
    AF = mybir.ActivationFunctionType

    nc = bacc.Bacc(
        "TRN2",
        target_bir_lowering=False,
        debug=False,
        num_devices=N_CORES,
    )

    host = nc.dram_tensor("host", [N_SH, DIM], f32, kind="ExternalInput").ap()
    guests = nc.dram_tensor("guests", [M_SH, DIM], f32, kind="ExternalInput").ap()
    Qp = nc.dram_tensor("wq", [DIM, DIM], f32, kind="ExternalInput").ap()
    Kp = nc.dram_tensor("wk", [DIM, DIM], f32, kind="ExternalInput").ap()
    Vp = nc.dram_tensor("wv", [DIM, DIM], f32, kind="ExternalInput").ap()
    out = nc.dram_tensor("out", [N_SH, DIM], f32, kind="ExternalOutput").ap()

    RG = [list(range(N_CORES))]
    NMO = N_GUEST // P        # 64 m-chunks of 128

    def AG(in_ap, out_ap):
        nc.gpsimd.collective_compute(
            "AllGather", mybir.AluOpType.bypass, replica_groups=RG,
            ins=[in_ap.opt()], outs=[out_ap.opt()],
        )

    with tile.TileContext(nc) as tc:
        with tc.tile_pool(name="persist", bufs=1) as persist, \
             tc.tile_pool(name="dram", bufs=1, space="DRAM") as dram:

            # ---- DRAM buffers ----
            g_dram = dram.tile([M_SH, DIM], bf16, name="g_dram")
            h_dram = dram.tile([N_SH, DIM], bf16, name="h_dram")
            k_in = [dram.tile([DIM, 256], bf16, name=f"k_in{q}") for q in range(4)]
            v_in = [dram.tile([512, DIM], bf16, name=f"v_in{h}") for h in range(2)]
            k_out = [dram.tile([N_CORES * DIM, 256], bf16, addr_space="Shared",
                               name=f"k_out{q}") for q in range(4)]
            v_out = [dram.tile([N_CORES * 512, DIM], bf16, addr_space="Shared",
                               name=f"v_out{h}") for h in range(2)]

            # ---- persistent SBUF ----
            qT = persist.tile([P, 8, N_SH], bf16, name="qT")          # [dout_i, dout_o, n]
            Psb = persist.tile([P, NMO, N_SH], bf16, name="Psb")      # [m_i, m_o, n] 8MB
            ones_sb = persist.tile([P, 1], bf16, name="ones_sb")
            id32 = persist.tile([P, P], f32, name="id32")
            rs_pad = persist.tile([P, N_SH], f32, name="rs_pad")
            rsT = persist.tile([P, 4], f32, name="rsT")
            recip = persist.tile([P, 4], f32, name="recip")

            nc.vector.memset(ones_sb, 1.0)
            nc.vector.memset(rs_pad, 0.0)
            make_identity(nc, id32)

            # pw2 outlives the pre-flash phase (used by value halves in-flash)
            with tc.tile_pool(name="pw2", bufs=1) as pw2:
                guestsT = pw2.tile([P, 8, M_SH], bf16, name="guestsT")
                V_sb = pw2.tile([P, 8, DIM], bf16, name="V_sb")
                v_loc = pw2.tile([P, 8, DIM], bf16, name="v_loc")

                # ============ pre-flash: projections + k AGs ============
                with tc.tile_pool(name="pw", bufs=1) as pw, \
                     tc.tile_pool(name="stage", bufs=4) as stage, \
                     tc.tile_pool(name="ps_mm", bufs=4, space="PSUM") as ps_mm:

                    def cast_to_dram(src, dst, rows):
                        for c in range(rows // P):
                            nat = stage.tile([P, DIM], f32, name="nat", tag="stage")
                            nc.sync.dma_start(nat, src[c * P:(c + 1) * P, :])
                            nbf = stage.tile([P, DIM], bf16, name="nbf", tag="stage_bf")
                            nc.vector.tensor_copy(out=nbf, in_=nat)
                            nc.sync.dma_start(dst[c * P:(c + 1) * P, :], nbf)

                    cast_to_dram(guests, g_dram, M_SH)
                    nc.sync.dma_start_transpose(guestsT, g_dram)
                    K_sb = pw.tile([P, 8, DIM], bf16, name="K_sb")
                    for c in range(8):
                        w_nat = stage.tile([P, DIM], f32, name="w_nat", tag="stage")
                        nc.sync.dma_start(w_nat, Kp[c * P:(c + 1) * P, :])
                        nc.vector.tensor_copy(out=K_sb[:, c, :], in_=w_nat)

                    # keyT shard halves; quarter chunks feed the k AGs
                    k_loc = pw.tile([P, 8, M_SH], bf16, name="k_loc")
                    for mh in range(2):
                        for dc in range(8):
                            mps = ps_mm.tile([P, 512], f32, name="mps", tag="mm")
                            for kc in range(8):
                                nc.tensor.matmul(
                                    mps,
                                    lhsT=K_sb[:, kc, dc * P:(dc + 1) * P],
                                    rhs=guestsT[:, kc, mh * 512:(mh + 1) * 512],
                                    start=(kc == 0), stop=(kc == 7),
                                )
                            nc.scalar.copy(
                                out=k_loc[:, dc, mh * 512:(mh + 1) * 512], in_=mps)
                        for q in (2 * mh, 2 * mh + 1):
                            nc.sync.dma_start(
                                k_in[q].rearrange("(ko ki) m -> ki ko m", ki=P),
                                k_loc[:, :, q * 256:(q + 1) * 256])
                            AG(k_in[q], k_out[q])

                    # hostT + qT
                    cast_to_dram(host, h_dram, N_SH)
                    hostT = pw.tile([P, 8, N_SH], bf16, name="hostT")
                    nc.sync.dma_start_transpose(hostT, h_dram)
                    Q_sb = pw.tile([P, 8, DIM], bf16, name="Q_sb")
                    for c in range(8):
                        w_nat = stage.tile([P, DIM], f32, name="w_nat2", tag="stage")
                        nc.sync.dma_start(w_nat, Qp[c * P:(c + 1) * P, :])
                        nc.vector.tensor_copy(out=Q_sb[:, c, :], in_=w_nat)
                    for dc in range(8):
                        qps = ps_mm.tile([P, N_SH], f32, name="qps", tag="mm")
                        for kc in range(8):
                            nc.tensor.matmul(
                                qps,
                                lhsT=Q_sb[:, kc, dc * P:(dc + 1) * P],
                                rhs=hostT[:, kc, :],
                                start=(kc == 0), stop=(kc == 7),
                            )
                        nc.scalar.copy(out=qT[:, dc, :], in_=qps)

                    for c in range(8):
                        w_nat = stage.tile([P, DIM], f32, name="w_nat3", tag="stage")
                        nc.sync.dma_start(w_nat, Vp[c * P:(c + 1) * P, :])
                        nc.vector.tensor_copy(out=V_sb[:, c, :], in_=w_nat)

                # views of the gathered buffers
                k_out_r = [k_out[q].rearrange("(b o i) m -> b i o m", o=8, i=P)
                           for q in range(4)]
                v_out_r = [v_out[h].rearrange("(b t i) d -> b i t d", t=4, i=P)
                           for h in range(2)]

                # ============ flash: S sweep + value halves + PV ============
                with tc.tile_pool(name="kvp", bufs=1) as kvp, \
                     tc.tile_pool(name="outp", bufs=8) as outp:

                    with tc.tile_pool(name="ps_st", bufs=2, space="PSUM") as ps_st, \
                         tc.tile_pool(name="ps_rs", bufs=1, space="PSUM") as ps_rs, \
                         tc.tile_pool(name="ps_vv", bufs=2, space="PSUM") as ps_vv:

                        def value_half(mh):
                            for mc in range(4 * mh, 4 * mh + 4):
                                for dh in range(2):
                                    mps = ps_vv.tile([P, 512], f32, name="mps2",
                                                     tag="vv")
                                    for kc in range(8):
                                        nc.tensor.matmul(
                                            mps,
                                            lhsT=guestsT[:, kc, mc * P:(mc + 1) * P],
                                            rhs=V_sb[:, kc, dh * 512:(dh + 1) * 512],
                                            start=(kc == 0), stop=(kc == 7),
                                        )
                                    nc.vector.tensor_copy(
                                        out=v_loc[:, mc, dh * 512:(dh + 1) * 512],
                                        in_=mps)
                            nc.sync.dma_start(
                                v_in[mh].rearrange("(mo mi) d -> mi mo d", mi=P),
                                v_loc[:, 4 * mh:4 * mh + 4, :])
                            AG(v_in[mh], v_out[mh])

                        rs_ps = ps_rs.tile([1, N_SH], f32, name="rs_ps")
                        t = 0
                        for q in range(4):
                            kts = []
                            for bb in range(8):
                                kT = kvp.tile([P, 8, 256], bf16, name="kT",
                                              tag="kT", bufs=10)
                                nc.sync.dma_start(kT, k_out_r[q][bb])
                                kts.append(kT)
                            for bb in range(8):
                                kT = kts[bb]
                                for j in range(2):
                                    mo = bb * 8 + q * 2 + j
                                    st = ps_st.tile([P, N_SH], f32, name="st", tag="st")
                                    for dc in range(8):
                                        nc.tensor.matmul(
                                            st,
                                            lhsT=kT[:, dc, j * P:(j + 1) * P],
                                            rhs=qT[:, dc, :],
                                            start=(dc == 0), stop=(dc == 7),
                                        )
                                    nc.scalar.activation(
                                        Psb[:, mo, :], st, AF.Exp, scale=1.0 / 32.0)
                                    nc.tensor.matmul(
                                        rs_ps, lhsT=ones_sb, rhs=Psb[:, mo, :],
                                        start=(t == 0), stop=(t == NMO - 1),
                                    )
                                    t += 1
                            if q == 0:
                                value_half(0)
                            elif q == 1:
                                value_half(1)
                        # rowsum [1, n] -> per-partition [n_chunk, 1] via PE transpose
                        nc.vector.tensor_copy(out=rs_pad[0:1, :], in_=rs_ps)
                        for c in range(4):
                            tp = ps_st.tile([P, P], f32, name="tp", tag="st")
                            nc.tensor.transpose(tp, rs_pad[:, c * P:(c + 1) * P], id32)
                            nc.vector.tensor_copy(out=rsT[:, c:c + 1], in_=tp[:, 0:1])
                        nc.vector.reciprocal(recip, rsT)

                    # ---- PV sweep (full dout, 8 PSUM banks) ----
                    with tc.tile_pool(name="ps_o", bufs=8, space="PSUM") as ps_o:
                        o_t = [[ps_o.tile([P, 512], f32, name=f"o_{c}_{hh}", tag="o")
                                for hh in range(2)] for c in range(4)]
                        t = 0
                        for h in range(2):
                            for bb in range(8):
                                vt = kvp.tile([P, 4, DIM], bf16, name="vt",
                                              tag="vt", bufs=3)
                                nc.sync.dma_start(vt, v_out_r[h][bb])
                                for j in range(4):
                                    mo = bb * 8 + h * 4 + j
                                    for c in range(4):
                                        for hh in range(2):
                                            nc.tensor.matmul(
                                                o_t[c][hh],
                                                lhsT=Psb[:, mo, c * P:(c + 1) * P],
                                                rhs=vt[:, j, hh * 512:(hh + 1) * 512],
                                                start=(t == 0), stop=(t == NMO - 1),
                                            )
                                    t += 1
                        for c in range(4):
                            for hh in range(2):
                                ot = outp.tile([P, 512], f32, name="ot", tag="ot")
                                nc.vector.tensor_scalar_mul(
                                    ot, o_t[c][hh], recip[:, c:c + 1])
                                nc.sync.dma_start(
                                    out[c * P:(c + 1) * P, hh * 512:(hh + 1) * 512], ot)

    nc.compile()
    return nc


def _get_nc():
    if "nc" not in _CACHE:
        _CACHE["nc"] = _build()
    return _CACHE["nc"]


def kernel(features_host, features_guests, Q, K, V):
    from concourse.bass_utils import run_bass_kernel_spmd

    nc = _get_nc()

    fh = np.ascontiguousarray(np.asarray(features_host, dtype=np.float32))
    fg = np.ascontiguousarray(np.asarray(features_guests, dtype=np.float32))
    Qn = np.ascontiguousarray(np.asarray(Q, dtype=np.float32))
    Kn = np.ascontiguousarray(np.asarray(K, dtype=np.float32))
    Vn = np.ascontiguousarray(np.asarray(V, dtype=np.float32))

    in_maps = []
    for c in range(N_CORES):
        in_maps.append({
            "host": np.ascontiguousarray(fh[c * N_SH:(c + 1) * N_SH]),
            "guests": np.ascontiguousarray(fg[c * M_SH:(c + 1) * M_SH]),
            "wq": Qn, "wk": Kn, "wv": Vn,
        })

    res = run_bass_kernel_spmd(nc, in_maps, core_ids=list(range(N_CORES)))
    outs = [np.asarray(res.results[c]["out"]) for c in range(N_CORES)]
    return np.concatenate(outs, axis=0).astype(np.float32)


# revision 11
# speedup vs baseline: 1.1169x; 1.1169x over previous
"""Distributed attention kernel for 8 Trainium2 NeuronCores.

reference:
    query = features_host @ Q          # [4096, 1024]
    key   = features_guests @ K        # [8192, 1024]
    value = features_guests @ V        # [8192, 1024]
    att   = softmax(query @ key.T / 32, axis=1)
    out   = att @ value                # [4096, 1024]

Sharding: host rows (N=4096) split across 8 cores (512 each). Guest rows
(M=8192) split across 8 cores (1024 each) for the key/value projections.
keyT is all-gathered as bf16 in two m-half chunks (so the S sweep starts
as soon as the first half lands); value as one 16MB gather (collectives
have a large per-op floor, and PV starts late enough to cover it).

Per-core pipeline (bf16 matmuls, fp32 PSUM accumulation):
  guests -> guestsT (bf16 PE transposes) -> keyT halves -> AG k0, AG k1
  hostT/qT and value shard while AGs fly -> AG v
  S sweep over m: S = keyT_blk.T @ qT, exp on ScalarE (scale=1/32) into a
    persistent bf16 P matrix, rowsum via ones-matmul in one PSUM bank
  PV sweep: O[n, 0:1024] accumulated across all m in all 8 PSUM banks
  divide by rowsum (PE-transposed to per-partition) and write out.
"""

import sys

for _p in ("/opt/trn_rl_repo", "/root/.axon_site/_ro/trn_rl_repo"):
    if _p not in sys.path:
        sys.path.insert(0, _p)

import numpy as np

N_HOST = 4096
N_GUEST = 8192
DIM = 1024
N_CORES = 8
N_SH = N_HOST // N_CORES      # 512 host rows per core
M_SH = N_GUEST // N_CORES     # 1024 guest rows per core
P = 128

_CACHE = {}


def _build():
    import concourse.bass as bass  # noqa: F401
    import concourse.mybir as mybir
    import concourse.tile as tile
    from concourse import bacc
    from concourse.masks import make_identity

    f32 = mybir.dt.float32
    bf16 = mybir.dt.bfloat16
    AF = mybir.ActivationFunctionType

    nc = bacc.Bacc(
        "TRN2",
        target_bir_lowering=False,
        debug=False,
        num_devices=N_CORES,
    )

    host = nc.dram_tensor("host", [N_SH, DIM], f32, kind="ExternalInput").ap()
    guests = nc.dram_tensor("guests", [M_SH, DIM], f32, kind="ExternalInput").ap()
    Qp = nc.dram_tensor("wq", [DIM, DIM], f32, kind="ExternalInput").ap()
    Kp = nc.dram_tensor("wk", [DIM, DIM], f32, kind="ExternalInput").ap()
    Vp = nc.dram_tensor("wv", [DIM, DIM], f32, kind="ExternalInput").ap()
    out = nc.dram_tensor("out", [N_SH, DIM], f32, kind="ExternalOutput").ap()

    RG = [list(range(N_CORES))]
    NMO = N_GUEST // P        # 64 m-chunks of 128

    def AG(in_ap, out_ap):
        nc.gpsimd.collective_compute(
            "AllGather", mybir.AluOpType.bypass, replica_groups=RG,
            ins=[in_ap.opt()], outs=[out_ap.opt()],
        )

    with tile.TileContext(nc) as tc:
        with tc.tile_pool(name="persist", bufs=1) as persist, \
             tc.tile_pool(name="dram", bufs=1, space="DRAM") as dram:

            # ---- DRAM collective buffers (bf16) ----
            k_in = [dram.tile([DIM, 512], bf16, name=f"k_in{h}") for h in range(2)]
            v_in = dram.tile([M_SH, DIM], bf16, name="v_in")
            k_out = [dram.tile([N_CORES * DIM, 512], bf16, addr_space="Shared",
                               name=f"k_out{h}") for h in range(2)]
            v_out = dram.tile([N_GUEST, DIM], bf16, addr_space="Shared", name="v_out")

            # ---- persistent SBUF ----
            qT = persist.tile([P, 8, N_SH], bf16, name="qT")          # [dout_i, dout_o, n]
            Psb = persist.tile([P, NMO, N_SH], bf16, name="Psb")      # [m_i, m_o, n] 8MB
            ones_sb = persist.tile([P, 1], bf16, name="ones_sb")
            identity = persist.tile([P, P], bf16, name="identity")
            id32 = persist.tile([P, P], f32, name="id32")
            rs_pad = persist.tile([P, N_SH], f32, name="rs_pad")
            rsT = persist.tile([P, 4], f32, name="rsT")
            recip = persist.tile([P, 4], f32, name="recip")

            nc.vector.memset(ones_sb, 1.0)
            nc.vector.memset(rs_pad, 0.0)
            make_identity(nc, identity)
            make_identity(nc, id32)

            # ============ pre-flash: projections + AGs ============
            with tc.tile_pool(name="pw", bufs=1) as pw, \
                 tc.tile_pool(name="stage", bufs=4) as stage, \
                 tc.tile_pool(name="ps_tr", bufs=2, space="PSUM") as ps_tr, \
                 tc.tile_pool(name="ps_mm", bufs=4, space="PSUM") as ps_mm:

                guestsT = pw.tile([P, 8, M_SH], bf16, name="guestsT")  # [din_i, din_o, m]

                def load_transpose(src_rows, dst, col):
                    nat = stage.tile([P, DIM], f32, name="nat", tag="stage")
                    nc.sync.dma_start(nat, src_rows)
                    nbf = stage.tile([P, DIM], bf16, name="nbf", tag="stage_bf")
                    nc.vector.tensor_copy(out=nbf, in_=nat)
                    for d in range(8):
                        tps = ps_tr.tile([P, P], bf16, name="tps", tag="tr")
                        nc.tensor.transpose(tps, nbf[:, d * P:(d + 1) * P], identity)
                        nc.vector.tensor_copy(out=dst[:, d, col:col + P], in_=tps)

                for c in range(M_SH // P):
                    load_transpose(guests[c * P:(c + 1) * P, :], guestsT, c * P)
                K_sb = pw.tile([P, 8, DIM], bf16, name="K_sb")
                for c in range(8):
                    w_nat = stage.tile([P, DIM], f32, name="w_nat", tag="stage")
                    nc.sync.dma_start(w_nat, Kp[c * P:(c + 1) * P, :])
                    nc.vector.tensor_copy(out=K_sb[:, c, :], in_=w_nat)

                # keyT shard [dout, m_loc]; m-half chunks feed the k AGs early
                k_loc = pw.tile([P, 8, M_SH], bf16, name="k_loc")
                for mh in range(2):
                    for dc in range(8):
                        mps = ps_mm.tile([P, 512], f32, name="mps", tag="mm")
                        for kc in range(8):
                            nc.tensor.matmul(
                                mps,
                                lhsT=K_sb[:, kc, dc * P:(dc + 1) * P],
                                rhs=guestsT[:, kc, mh * 512:(mh + 1) * 512],
                                start=(kc == 0), stop=(kc == 7),
                            )
                        nc.scalar.copy(out=k_loc[:, dc, mh * 512:(mh + 1) * 512], in_=mps)
                    nc.sync.dma_start(
                        k_in[mh].rearrange("(ko ki) m -> ki ko m", ki=P),
                        k_loc[:, :, mh * 512:(mh + 1) * 512])
                    AG(k_in[mh], k_out[mh])

                # hostT + qT (overlaps k AGs)
                Q_sb = pw.tile([P, 8, DIM], bf16, name="Q_sb")
                for c in range(8):
                    w_nat = stage.tile([P, DIM], f32, name="w_nat2", tag="stage")
                    nc.sync.dma_start(w_nat, Qp[c * P:(c + 1) * P, :])
                    nc.vector.tensor_copy(out=Q_sb[:, c, :], in_=w_nat)
                hostT = pw.tile([P, 8, N_SH], bf16, name="hostT")     # [din_i, din_o, n]
                for c in range(N_SH // P):
                    load_transpose(host[c * P:(c + 1) * P, :], hostT, c * P)
                for dc in range(8):
                    qps = ps_mm.tile([P, N_SH], f32, name="qps", tag="mm")
                    for kc in range(8):
                        nc.tensor.matmul(
                            qps,
                            lhsT=Q_sb[:, kc, dc * P:(dc + 1) * P],
                            rhs=hostT[:, kc, :],
                            start=(kc == 0), stop=(kc == 7),
                        )
                    nc.scalar.copy(out=qT[:, dc, :], in_=qps)

                # value shard [m_loc, dout] -> one 16MB all-gather
                V_sb = pw.tile([P, 8, DIM], bf16, name="V_sb")
                for c in range(8):
                    w_nat = stage.tile([P, DIM], f32, name="w_nat3", tag="stage")
                    nc.sync.dma_start(w_nat, Vp[c * P:(c + 1) * P, :])
                    nc.vector.tensor_copy(out=V_sb[:, c, :], in_=w_nat)
                v_loc = pw.tile([P, 8, DIM], bf16, name="v_loc")      # [m_i, m_o, dout]
                for mc in range(8):
                    for dh in range(2):
                        mps = ps_mm.tile([P, 512], f32, name="mps2", tag="mm")
                        for kc in range(8):
                            nc.tensor.matmul(
                                mps,
                                lhsT=guestsT[:, kc, mc * P:(mc + 1) * P],
                                rhs=V_sb[:, kc, dh * 512:(dh + 1) * 512],
                                start=(kc == 0), stop=(kc == 7),
                            )
                        nc.vector.tensor_copy(
                            out=v_loc[:, mc, dh * 512:(dh + 1) * 512], in_=mps)
                nc.sync.dma_start(
                    v_in.rearrange("(mo mi) d -> mi mo d", mi=P), v_loc)
                AG(v_in, v_out)

            # views of the gathered buffers
            k_out_r = [k_out[h].rearrange("(b o i) m -> b i o m", o=8, i=P)
                       for h in range(2)]
            v_out_r = v_out.rearrange("(b t i) d -> b i t d", t=8, i=P)

            # ============ flash: S sweep then PV sweep ============
            with tc.tile_pool(name="kvp", bufs=1) as kvp, \
                 tc.tile_pool(name="outp", bufs=8) as outp:
                with tc.tile_pool(name="ps_st", bufs=2, space="PSUM") as ps_st, \
                     tc.tile_pool(name="ps_rs", bufs=1, space="PSUM") as ps_rs:
                    rs_ps = ps_rs.tile([1, N_SH], f32, name="rs_ps")
                    t = 0
                    for h in range(2):
                        kts = []
                        for bb in range(8):
                            kT = kvp.tile([P, 8, 512], bf16, name="kT",
                                          tag="kT", bufs=10)
                            nc.sync.dma_start(kT, k_out_r[h][bb])
                            kts.append(kT)
                        for bb in range(8):
                            kT = kts[bb]
                            for j in range(4):
                                mo = bb * 8 + h * 4 + j
                                st = ps_st.tile([P, N_SH], f32, name="st", tag="st")
                                for dc in range(8):
                                    nc.tensor.matmul(
                                        st,
                                        lhsT=kT[:, dc, j * P:(j + 1) * P],
                                        rhs=qT[:, dc, :],
                                        start=(dc == 0), stop=(dc == 7),
                                    )
                                nc.scalar.activation(
                                    Psb[:, mo, :], st, AF.Exp, scale=1.0 / 32.0)
                                nc.tensor.matmul(
                                    rs_ps, lhsT=ones_sb, rhs=Psb[:, mo, :],
                                    start=(t == 0), stop=(t == NMO - 1),
                                )
                                t += 1
                    # rowsum [1, n] -> per-partition [n_chunk, 1] via PE transpose
                    nc.vector.tensor_copy(out=rs_pad[0:1, :], in_=rs_ps)
                    for c in range(4):
                        tp = ps_st.tile([P, P], f32, name="tp", tag="st")
                        nc.tensor.transpose(tp, rs_pad[:, c * P:(c + 1) * P], id32)
                        nc.vector.tensor_copy(out=rsT[:, c:c + 1], in_=tp[:, 0:1])
                    nc.vector.reciprocal(recip, rsT)

                # ---- PV sweep (full dout, 8 PSUM banks) ----
                with tc.tile_pool(name="ps_o", bufs=8, space="PSUM") as ps_o:
                    o_t = [[ps_o.tile([P, 512], f32, name=f"o_{c}_{hh}", tag="o")
                            for hh in range(2)] for c in range(4)]
                    t = 0
                    for bb in range(8):
                        for hl in range(2):
                            vt = kvp.tile([P, 4, DIM], bf16, name="vt",
                                          tag="vt", bufs=4)
                            nc.sync.dma_start(vt, v_out_r[bb, :, 4 * hl:4 * hl + 4])
                            for j in range(4):
                                mo = bb * 8 + hl * 4 + j
                                for c in range(4):
                                    for hh in range(2):
                                        nc.tensor.matmul(
                                            o_t[c][hh],
                                            lhsT=Psb[:, mo, c * P:(c + 1) * P],
                                            rhs=vt[:, j, hh * 512:(hh + 1) * 512],
                                            start=(t == 0), stop=(t == NMO - 1),
                                        )
                                t += 1
                    for c in range(4):
                        for hh in range(2):
                            ot = outp.tile([P, 512], f32, name="ot", tag="ot")
                            nc.vector.tensor_scalar_mul(
                                ot, o_t[c][hh], recip[:, c:c + 1])
                            nc.sync.dma_start(
                                out[c * P:(c + 1) * P, hh * 512:(hh + 1) * 512], ot)

    nc.compile()
    return nc


def _get_nc():
    if "nc" not in _CACHE:
        _CACHE["nc"] = _build()
    return _CACHE["nc"]


def kernel(features_host, features_guests, Q, K, V):
    from concourse.bass_utils import run_bass_kernel_spmd

    nc = _get_nc()

    fh = np.ascontiguousarray(np.asarray(features_host, dtype=np.float32))
    fg = np.ascontiguousarray(np.asarray(features_guests, dtype=np.float32))
    Qn = np.ascontiguousarray(np.asarray(Q, dtype=np.float32))
    Kn = np.ascontiguousarray(np.asarray(K, dtype=np.float32))
    Vn = np.ascontiguousarray(np.asarray(V, dtype=np.float32))

    in_maps = []
    for c in range(N_CORES):
        in_maps.append({
            "host": np.ascontiguousarray(fh[c * N_SH:(c + 1) * N_SH]),
            "guests": np.ascontiguousarray(fg[c * M_SH:(c + 1) * M_SH]),
            "wq": Qn, "wk": Kn, "wv": Vn,
        })

    res = run_bass_kernel_spmd(nc, in_maps, core_ids=list(range(N_CORES)))
    outs = [np.asarray(res.results[c]["out"]) for c in range(N_CORES)]
    return np.concatenate(outs, axis=0).astype(np.float32)
